# revision 18
# baseline (speedup 1.0000x reference)
"""ChronosMOE FeedForward on 8 Trainium2 NeuronCores.

Strategy (expert-parallel, sparse v5 — bf16 datapath):
  - Host computes router top-2 SELECTION only (the dispatch plan), gathers
    each expert's tokens owner-sorted, and ships core e its expert weights
    (re-blocked, bf16) plus gathered activations (f32 for exact on-device
    router recompute; converted to bf16 on device for the FFN matmuls).
  - Core e re-computes router logits for its gathered tokens in exact f32
    and derives top-2 softmax combine weights numerically (min top2/top3
    logit gap for this input is ~4e-4, so f32 exactness is required to
    reproduce the host's selection).
  - All FFN matmuls (expert g/u/down, shared g/u/down, merge) run in bf16
    (same PE rate as f32r, half the HBM/SBUF footprint). wg/wu/wd are
    SBUF-resident across both token batches; gs/us stream through a pool.
  - Per batch: one weight-stationary sweep with fused down-projection
    (persistent PSUM accumulators, down-proj lagged one I-tile for
    pipelining), then combine-weight scaling and an 8-core AllToAll of the
    compact bf16 outputs (48 rows per (expert, owner) pair).
  - The shared-expert g/u + down and both merges run after the second
    sweep, covering the second AllToAll's latency.
  - Bulk unconditional DMAs ride the sync-engine ring; latency-critical
    conditional DMAs (a2a stores, a2a-out loads, y stores) ride the
    scalar-engine ring so they never queue behind pool-paced weight loads.
  - Core c returns output rows {c*128..} of each batch; host concatenates.
"""
import numpy as np
import ml_dtypes

import concourse.bass as bass
import concourse.mybir as mybir
import concourse.tile as tile
from concourse import bacc
from concourse.bass_utils import run_bass_kernel_spmd
from concourse.masks import make_identity

F32 = mybir.dt.float32
BF16 = mybir.dt.bfloat16
AF = mybir.ActivationFunctionType
OP = mybir.AluOpType
BF16NP = ml_dtypes.bfloat16

H = 1024          # hidden
E = 8             # experts
I = 1408          # moe intermediate
B, S = 2, 1024
T = B * S         # 2048 tokens
NCORES = 8
HC = H // 128     # 8 H-chunks
IC = I // 128     # 11 I-tiles
NB = 2            # token batches
TB = T // NB      # 1024 tokens per batch
SLOT = 48         # A2A slots per (expert, owner) pair (max observed 44)
CAP = SLOT * NCORES   # 384 gathered tokens per batch
CB = CAP // 128   # gathered token tiles per batch
SST = 256         # shared-expert tokens per core (2 x 128)
HN = H // 512     # 2 down-proj output column groups

_CACHE = {}


def _build():
    nc = bacc.Bacc("TRN2", target_bir_lowering=False, debug=False,
                   num_devices=NCORES)

    xg_d = [nc.dram_tensor(f"xg{b}", [128, HC, CAP], F32,
                           kind="ExternalInput") for b in range(NB)]
    xgb_d = [nc.dram_tensor(f"xgb{b}", [128, HC, CAP], BF16,
                            kind="ExternalInput") for b in range(NB)]
    xs_d = nc.dram_tensor("xs", [128, HC, SST], BF16, kind="ExternalInput")
    wr_d = nc.dram_tensor("wrT", [128, HC, E], F32, kind="ExternalInput")
    esel_d = nc.dram_tensor("esel", [128, E], F32, kind="ExternalInput")
    wg_d = nc.dram_tensor("wgB", [128, IC, HC, 128], BF16,
                          kind="ExternalInput")
    wu_d = nc.dram_tensor("wuB", [128, IC, HC, 128], BF16,
                          kind="ExternalInput")
    wgs_d = nc.dram_tensor("wgsB", [128, IC, HC, 128], BF16,
                           kind="ExternalInput")
    wus_d = nc.dram_tensor("wusB", [128, IC, HC, 128], BF16,
                           kind="ExternalInput")
    wd_d = nc.dram_tensor("wdB", [128, IC, H], BF16, kind="ExternalInput")
    wds_d = nc.dram_tensor("wdsB", [128, IC, H], BF16, kind="ExternalInput")
    sm_d = nc.dram_tensor("smB", [NB, 128, CB, 128], BF16,
                          kind="ExternalInput")
    y_d = nc.dram_tensor("y", [SST, H], F32, kind="ExternalOutput")

    with tile.TileContext(nc) as tc:
        with (
            tc.tile_pool(name="wres", bufs=1) as wres,
            tc.tile_pool(name="xgf", bufs=1) as xgfp,
            tc.tile_pool(name="wsh", bufs=8) as wshp,
            tc.tile_pool(name="act", bufs=1) as act,
            tc.tile_pool(name="small", bufs=2) as small,
            tc.tile_pool(name="sgp", bufs=2) as sgp,
            tc.tile_pool(name="htmp", bufs=3) as htmp,
            tc.tile_pool(name="osb", bufs=3) as osb,
            tc.tile_pool(name="fin", bufs=3) as fin,
            tc.tile_pool(name="yp", bufs=2) as ypool,
            tc.tile_pool(name="psA", bufs=1, space="PSUM") as psA,
            tc.tile_pool(name="psB", bufs=1, space="PSUM") as psB,
            tc.tile_pool(name="dram", bufs=1, space="DRAM") as dram,
        ):
            a2a_in = [dram.tile([CAP, H], BF16, tag=f"ai{b}", name=f"ai{b}")
                      for b in range(NB)]
            a2a_out = [dram.tile([CAP, H], BF16, tag=f"ao{b}", name=f"ao{b}")
                       for b in range(NB)]

            # ---- bulk unconditional loads (sync ring), in consumption order
            xgb = []
            t = act.tile([128, HC, CAP], BF16, tag="xgb0", name="xgb0")
            nc.sync.dma_start(t[:], xgb_d[0][:])
            xgb.append(t)
            wrT_sb = wres.tile([128, HC, E], F32, tag="wrT")
            nc.sync.dma_start(wrT_sb[:], wr_d[:])
            esel_sb = wres.tile([128, E], F32, tag="esel")
            nc.sync.dma_start(esel_sb[:], esel_d[:])
            ident8 = wres.tile([8, 8], F32, tag="ident8")
            make_identity(nc, ident8[:])

            wg_sb = wres.tile([128, IC, HC, 128], BF16, tag="wg")
            wu_sb = wres.tile([128, IC, HC, 128], BF16, tag="wu")
            wd_sb = wres.tile([128, IC, H], BF16, tag="wd")
            # small leading groups so sweep(0) can start early
            groups = [(0, 1), (1, 3), (3, 6), (6, 9), (9, 11)]
            for gi, (i0, i1) in enumerate(groups):
                for dst, src in ((wg_sb, wg_d), (wu_sb, wu_d), (wd_sb, wd_d)):
                    nc.sync.dma_start(dst[:, i0:i1], src[:, i0:i1])
                if gi == 0:
                    # f32 router activations (consumed by the router matmuls
                    # interleaved a few I-tiles into each sweep)
                    t = act.tile([128, HC, CAP], BF16, tag="xgb1",
                                 name="xgb1")
                    nc.sync.dma_start(t[:], xgb_d[1][:])
                    xgb.append(t)
                    xgf = []
                    for b in range(NB):
                        t = xgfp.tile([128, HC, CAP], F32, tag=f"xgf{b}",
                                      name=f"xgf{b}")
                        nc.sync.dma_start(t[:], xg_d[b][:])
                        xgf.append(t)
            xs_sb = act.tile([128, HC, SST], BF16, tag="xs")
            nc.sync.dma_start(xs_sb[:], xs_d[:])
            sm_sb = act.tile([128, NB, CB, 128], BF16, tag="sm")
            for b in range(NB):
                nc.sync.dma_start(sm_sb[:, b], sm_d[b])
            wds_sb = wres.tile([128, IC, H], BF16, tag="wds")
            for i0, i1 in ((0, 6), (6, 11)):
                nc.sync.dma_start(wds_sb[:, i0:i1], wds_d[:, i0:i1])
            # shared-expert g/u weights stream (pool-paced WAR waits are fine
            # at the tail of the sync ring)
            wsh = {}
            for it in range(IC):
                for nm, src in (("gs", wgs_d), ("us", wus_d)):
                    t = wshp.tile([128, HC, 128], BF16, tag="wsh",
                                  name=f"wsh_{nm}{it}")
                    nc.sync.dma_start(t[:], src[:, it])
                    wsh[(nm, it)] = t

            def router_cw(b):
                """Exact-f32 top-2 softmax combine weight for this core's
                expert, per gathered token slot.  Identical math to v4."""
                lgT_ps = psA.tile([8, CAP], F32, tag="g_ps", name=f"lgT{b}")
                for hc in range(HC):
                    nc.tensor.matmul(lgT_ps[:], wrT_sb[:, hc, :],
                                     xgf[b][:, hc, :],
                                     start=(hc == 0), stop=(hc == HC - 1))
                lgT_sb = small.tile([8, CAP], F32, tag="lgTs",
                                    name=f"lgTs{b}")
                nc.vector.tensor_copy(lgT_sb[:], lgT_ps[:])
                lg = small.tile([128, CB, E], F32, tag="lg", name=f"lg{b}")
                for m4 in range(CB):
                    ltr_ps = psA.tile([128, 8], F32, tag="u_ps",
                                      name=f"ltr{b}_{m4}")
                    nc.tensor.transpose(
                        ltr_ps[:], lgT_sb[:, m4 * 128:(m4 + 1) * 128],
                        ident8[:])
                    nc.vector.tensor_copy(lg[:, m4, :], ltr_ps[:])
                m1 = small.tile([128, CB, 1], F32, tag="m1", name=f"m1{b}")
                nc.vector.tensor_reduce(m1[:], lg[:], axis=mybir.AxisListType.X,
                                        op=OP.max)
                m1b = m1[:].to_broadcast([128, CB, E])
                is1 = small.tile([128, CB, E], F32, tag="is1", name=f"is1{b}")
                nc.vector.tensor_tensor(is1[:], lg[:], m1b, OP.is_ge)
                lgm = small.tile([128, CB, E], F32, tag="lgm", name=f"lgm{b}")
                nc.vector.scalar_tensor_tensor(
                    lgm[:], is1[:], -1e30, lg[:], op0=OP.mult, op1=OP.add)
                m2 = small.tile([128, CB, 1], F32, tag="m2", name=f"m2{b}")
                nc.vector.tensor_reduce(m2[:], lgm[:], axis=mybir.AxisListType.X,
                                        op=OP.max)
                dd = small.tile([128, CB, E], F32, tag="dd", name=f"dd{b}")
                nc.vector.tensor_tensor(dd[:], lg[:], m1b, OP.subtract)
                ee = small.tile([128, CB, E], F32, tag="ee", name=f"ee{b}")
                nc.scalar.activation(ee[:], dd[:], AF.Exp)
                d2 = small.tile([128, CB, 1], F32, tag="d2", name=f"d2{b}")
                nc.vector.tensor_tensor(d2[:], m2[:], m1[:], OP.subtract)
                e2 = small.tile([128, CB, 1], F32, tag="e2", name=f"e2{b}")
                nc.scalar.activation(e2[:], d2[:], AF.Exp)
                den = small.tile([128, CB, 1], F32, tag="den", name=f"den{b}")
                nc.vector.tensor_scalar_add(den[:], e2[:], 1.0)
                rden = small.tile([128, CB, 1], F32, tag="rden",
                                  name=f"rden{b}")
                nc.vector.reciprocal(rden[:], den[:])
                mask = small.tile([128, CB, E], F32, tag="mask",
                                  name=f"mask{b}")
                nc.vector.tensor_tensor(mask[:], lg[:],
                                        m2[:].to_broadcast([128, CB, E]),
                                        OP.is_ge)
                cwa = small.tile([128, CB, E], F32, tag="cwa", name=f"cwa{b}")
                nc.vector.tensor_tensor(cwa[:], ee[:], mask[:], OP.mult)
                nc.vector.tensor_tensor(cwa[:], cwa[:],
                                        rden[:].to_broadcast([128, CB, E]),
                                        OP.mult)
                esel_b = esel_sb[:].unsqueeze(1).to_broadcast([128, CB, E])
                nc.vector.tensor_tensor(cwa[:], cwa[:], esel_b, OP.mult)
                cwt = small.tile([128, CB, 1], F32, tag=f"cw{b}",
                                 name=f"cw{b}")
                nc.vector.tensor_reduce(cwt[:], cwa[:], axis=mybir.AxisListType.X,
                                        op=OP.add)
                return cwt

            cw_g = [None, None]

            def sweep(b):
                """g/u + down-proj (lagged one I-tile) for batch b.
                Router for batch b is interleaved after I-tile 0's g/u so
                the PE is already warm and nothing blocks sweep start."""
                ob = [psB.tile([128, 512], F32, tag=f"oA{j}", name=f"ob{b}_{j}")
                      for j in range(2 * CB)]
                h_prev = None

                def down(it, h0):
                    for m in range(CB):
                        for hn in range(HN):
                            nc.tensor.matmul(
                                ob[m * HN + hn][:],
                                h0[:, m * 128:(m + 1) * 128],
                                wd_sb[:, it, hn * 512:(hn + 1) * 512],
                                start=(it == 0), stop=(it == IC - 1))

                for it in range(IC):
                    g_ps = psA.tile([128, CAP], F32, tag="g_ps",
                                    name=f"g{b}_{it}")
                    for hc in range(HC):
                        nc.tensor.matmul(g_ps[:], wg_sb[:, it, hc, :],
                                         xgb[b][:, hc, :],
                                         start=(hc == 0), stop=(hc == HC - 1))
                    sg = sgp.tile([128, CAP], F32, tag="sg",
                                  name=f"sg{b}_{it}")
                    nc.scalar.activation(sg[:], g_ps[:], AF.Silu)
                    u_ps = psA.tile([128, CAP], F32, tag="u_ps",
                                    name=f"u{b}_{it}")
                    for hc in range(HC):
                        nc.tensor.matmul(u_ps[:], wu_sb[:, it, hc, :],
                                         xgb[b][:, hc, :],
                                         start=(hc == 0), stop=(hc == HC - 1))
                    h0 = htmp.tile([128, CAP], BF16, tag="h0",
                                   name=f"h{b}_{it}")
                    nc.vector.tensor_tensor(h0[:], sg[:], u_ps[:], OP.mult)
                    if it == 2:
                        # after h0 so the psA WAR chain (silu/h0 reads) is
                        # already in the program when router reuses the tags;
                        # a few I-tiles in so the PE is warm and the f32
                        # activations have arrived
                        cw_g[b] = router_cw(b)
                    if h_prev is not None:
                        down(it - 1, h_prev)
                    h_prev = h0
                down(IC - 1, h_prev)
                # scale by combine weight, write compact bf16, exchange;
                # stores ride the scalar HWDGE ring (gpsimd SWDGE pays ~5us
                # of descriptor emission per 128-row store)
                for m in range(CB):
                    o_sb = osb.tile([128, H], BF16, tag="o_sb",
                                    name=f"osb{b}_{m}")
                    for hn in range(HN):
                        nc.vector.tensor_scalar_mul(
                            o_sb[:, hn * 512:(hn + 1) * 512],
                            ob[m * HN + hn][:], cw_g[b][:, m, :])
                    nc.scalar.dma_start(a2a_in[b][m * 128:(m + 1) * 128, :],
                                        o_sb[:])
                nc.gpsimd.collective_compute(
                    "AllToAll", OP.bypass,
                    replica_groups=[list(range(NCORES))],
                    ins=[a2a_in[b][:].opt()],
                    outs=[a2a_out[b][:].opt()],
                )

            sweep(0)
            sweep(1)

            # a2a output loads ride the tail of the sync ring: their waits on
            # collective completion cannot block any other engine's stream
            rc = {}
            for b in range(NB):
                for rk in range(CB):
                    t = fin.tile([128, H], BF16, tag="rc", name=f"rc{b}_{rk}")
                    nc.sync.dma_start(
                        t[:], a2a_out[b][rk * 128:(rk + 1) * 128, :])
                    rc[(b, rk)] = t

            # ---- merge batch 0 right after sweep(1): keeps the PE warm and
            # runs long before anything needs it ----
            y_ps = {}

            def merge(b):
                for hn in range(HN):
                    hsl = slice(hn * 512, (hn + 1) * 512)
                    yp = psB.tile([128, 512], F32, tag=f"oA{2 * b + hn}",
                                  name=f"y_ps{b}_{hn}")
                    for rk in range(CB):
                        nc.tensor.matmul(yp[:], sm_sb[:, b, rk, :],
                                         rc[(b, rk)][:, hsl],
                                         start=(rk == 0), stop=(rk == CB - 1))
                    y_ps[(b, hn)] = yp

            merge(0)

            # ---- shared expert g/u (covers a2a latency) ----
            hs_sb = act.tile([128, IC, SST], BF16, tag="hs")
            for it in range(IC):
                gs_ps = psA.tile([128, SST], F32, tag="g_ps",
                                 name=f"gs_{it}")
                for hc in range(HC):
                    nc.tensor.matmul(gs_ps[:], wsh[("gs", it)][:, hc, :],
                                     xs_sb[:, hc, :],
                                     start=(hc == 0), stop=(hc == HC - 1))
                sgs = sgp.tile([128, SST], F32, tag="sgs", name=f"sgs_{it}")
                nc.scalar.activation(sgs[:], gs_ps[:], AF.Silu)
                us_ps = psA.tile([128, SST], F32, tag="u_ps",
                                 name=f"us_{it}")
                for hc in range(HC):
                    nc.tensor.matmul(us_ps[:], wsh[("us", it)][:, hc, :],
                                     xs_sb[:, hc, :],
                                     start=(hc == 0), stop=(hc == HC - 1))
                nc.vector.tensor_tensor(hs_sb[:, it, :], sgs[:], us_ps[:],
                                        OP.mult)

            # ---- shared down-proj ----
            s_out = act.tile([128, NB, H], F32, tag="s_out")
            for hn in range(HN):
                hsl = slice(hn * 512, (hn + 1) * 512)
                s_ps = [psA.tile([128, 512], F32, tag=("g_ps", "u_ps")[m],
                                 name=f"s_ps{m}_{hn}") for m in range(NB)]
                for it in range(IC):
                    for m in range(NB):
                        nc.tensor.matmul(s_ps[m][:],
                                         hs_sb[:, it, m * 128:(m + 1) * 128],
                                         wds_sb[:, it, hsl],
                                         start=(it == 0), stop=(it == IC - 1))
                for m in range(NB):
                    nc.scalar.copy(s_out[:, m, hsl], s_ps[m][:])

            # ---- finalize batch 0, then batch 1 ----
            def finalize(b):
                y_sb = ypool.tile([128, H], F32, tag="y_sb", name=f"ysb{b}")
                for hn in range(HN):
                    hsl = slice(hn * 512, (hn + 1) * 512)
                    nc.vector.tensor_tensor(y_sb[:, hsl], y_ps[(b, hn)][:],
                                            s_out[:, b, hsl], OP.add)
                nc.scalar.dma_start(y_d[b * 128:(b + 1) * 128, :], y_sb[:])

            merge(1)
            finalize(0)
            finalize(1)

    nc.compile()
    return nc


def _get_nc():
    if "nc" not in _CACHE:
        _CACHE["nc"] = _build()
    return _CACHE["nc"]


def _reblock_gu(w):
    # [H, I] -> [128, IC, HC, 128] bf16: [q, it, hc, p] = w[hc*128+q, it*128+p]
    return np.ascontiguousarray(
        w.reshape(HC, 128, IC, 128).transpose(1, 2, 0, 3)).astype(BF16NP)


def _reblock_d(w):
    # [I, H] -> [128, IC, H] bf16: [k, it, h] = w[it*128+k, h]
    return np.ascontiguousarray(
        w.reshape(IC, 128, H).transpose(1, 0, 2)).astype(BF16NP)


def _pack_pm(a):
    # [H, N] -> [128, HC, N]: [p, hc, n] = a[hc*128+p, n]
    return np.ascontiguousarray(a.reshape(HC, 128, -1).transpose(1, 0, 2))


def make_in_maps(x, w_router, wg, wu, wd, wg_s, wu_s, wd_s):
    xf = x.reshape(T, H)
    xT = np.ascontiguousarray(xf.T)

    # host-side dispatch plan: top-2 selection per token
    logits = xf @ w_router.T                      # [T, E]
    part = np.argpartition(-logits, 2, axis=1)[:, :2]   # top-2 expert ids

    wrT = _pack_pm(np.ascontiguousarray(w_router.T))    # [128, HC, E] f32
    wgsB = _reblock_gu(wg_s)
    wusB = _reblock_gu(wu_s)
    wdsB = _reblock_d(wd_s)

    # dispatch tables: for (batch, expert) owner-sorted slot assignment
    gsel = np.zeros((NB, NCORES, CAP), np.int64)      # gathered token ids
    smT = np.zeros((NB, NCORES, CAP, 128), np.float32)  # receiver merge mats
    for b in range(NB):
        sel_b = part[b * TB:(b + 1) * TB]
        for e in range(NCORES):
            sel = np.where((sel_b == e).any(axis=1))[0]   # tokens picking e
            gsel[b, e, :] = b * TB                        # pad default
            for o in range(NCORES):
                grp = sel[(sel // 128) == o]
                n = len(grp)
                assert n <= SLOT, f"slot overflow: {n} > {SLOT}"
                gsel[b, e, o * SLOT:o * SLOT + n] = b * TB + grp
                # receiver o's merge matrix: recv row e*SLOT+k -> local row
                smT[b, o, e * SLOT + np.arange(n), grp - o * 128] = 1.0
    in_maps = []
    for c in range(NCORES):
        xsT = np.concatenate([xT[:, c * 128:(c + 1) * 128],
                              xT[:, TB + c * 128:TB + (c + 1) * 128]], axis=1)
        m = {
            "xs": _pack_pm(xsT).astype(BF16NP),
            "wrT": wrT,
            "wgB": _reblock_gu(wg[c]),
            "wuB": _reblock_gu(wu[c]),
            "wdB": _reblock_d(wd[c]),
            "wgsB": wgsB,
            "wusB": wusB,
            "wdsB": wdsB,
        }
        esel = np.zeros((128, E), np.float32)
        esel[:, c] = 1.0
        m["esel"] = esel
        for b in range(NB):
            xgc = _pack_pm(np.ascontiguousarray(xT[:, gsel[b, c]]))
            m[f"xg{b}"] = xgc
            m[f"xgb{b}"] = xgc.astype(BF16NP)
        m["smB"] = np.ascontiguousarray(
            smT[:, c].reshape(NB, CB, 128, 128).transpose(0, 2, 1, 3)
        ).astype(BF16NP)
        in_maps.append(m)
    return in_maps


def kernel(x, w_router, wg, wu, wd, wg_s, wu_s, wd_s):
    x = np.asarray(x, dtype=np.float32)
    w_router = np.asarray(w_router, dtype=np.float32)
    wg = np.asarray(wg, dtype=np.float32)
    wu = np.asarray(wu, dtype=np.float32)
    wd = np.asarray(wd, dtype=np.float32)
    wg_s = np.asarray(wg_s, dtype=np.float32)
    wu_s = np.asarray(wu_s, dtype=np.float32)
    wd_s = np.asarray(wd_s, dtype=np.float32)

    nc = _get_nc()
    in_maps = make_in_maps(x, w_router, wg, wu, wd, wg_s, wu_s, wd_s)
    res = run_bass_kernel_spmd(nc, in_maps, list(range(NCORES)))

    y = np.zeros((T, H), np.float32)
    for c in range(NCORES):
        yc = res.results[c]["y"]
        for b in range(NB):
            y[b * TB + c * 128: b * TB + (c + 1) * 128] = \
                yc[b * 128:(b + 1) * 128]
    return y.reshape(B, S, H)


# revision 21
# speedup vs baseline: 1.0468x; 1.0468x over previous
"""ChronosMOE FeedForward on 8 Trainium2 NeuronCores.

Strategy (expert-parallel, sparse v5 — bf16 datapath):
  - Host computes router top-2 SELECTION only (the dispatch plan), gathers
    each expert's tokens owner-sorted, and ships core e its expert weights
    (re-blocked, bf16) plus gathered activations (f32 for exact on-device
    router recompute; converted to bf16 on device for the FFN matmuls).
  - Core e re-computes router logits for its gathered tokens in exact f32
    and derives top-2 softmax combine weights numerically (min top2/top3
    logit gap for this input is ~4e-4, so f32 exactness is required to
    reproduce the host's selection).
  - All FFN matmuls (expert g/u/down, shared g/u/down, merge) run in bf16
    (same PE rate as f32r, half the HBM/SBUF footprint). wg/wu/wd are
    SBUF-resident across both token batches; gs/us stream through a pool.
  - Per batch: one weight-stationary sweep with fused down-projection
    (persistent PSUM accumulators, down-proj lagged one I-tile for
    pipelining), then combine-weight scaling and an 8-core AllToAll of the
    compact bf16 outputs (48 rows per (expert, owner) pair).
  - The shared-expert g/u + down and both merges run after the second
    sweep, covering the second AllToAll's latency.
  - Bulk unconditional DMAs ride the sync-engine ring; latency-critical
    conditional DMAs (a2a stores, a2a-out loads, y stores) ride the
    scalar-engine ring so they never queue behind pool-paced weight loads.
  - Core c returns output rows {c*128..} of each batch; host concatenates.
"""
import numpy as np
import ml_dtypes

import concourse.bass as bass
import concourse.mybir as mybir
import concourse.tile as tile
from concourse import bacc
from concourse.bass_utils import run_bass_kernel_spmd
from concourse.masks import make_identity

F32 = mybir.dt.float32
BF16 = mybir.dt.bfloat16
AF = mybir.ActivationFunctionType
OP = mybir.AluOpType
BF16NP = ml_dtypes.bfloat16

H = 1024          # hidden
E = 8             # experts
I = 1408          # moe intermediate
B, S = 2, 1024
T = B * S         # 2048 tokens
NCORES = 8
HC = H // 128     # 8 H-chunks
IC = I // 128     # 11 I-tiles
NB = 2            # token batches
TB = T // NB      # 1024 tokens per batch
SLOT = 48         # A2A slots per (expert, owner) pair (max observed 44)
CAP = SLOT * NCORES   # 384 gathered tokens per batch
CB = CAP // 128   # gathered token tiles per batch
SST = 256         # shared-expert tokens per core (2 x 128)
HN = H // 512     # 2 down-proj output column groups

_CACHE = {}


def _build():
    nc = bacc.Bacc("TRN2", target_bir_lowering=False, debug=False,
                   num_devices=NCORES)

    xg_d = [nc.dram_tensor(f"xg{b}", [128, HC, CAP], F32,
                           kind="ExternalInput") for b in range(NB)]
    xgb_d = [nc.dram_tensor(f"xgb{b}", [128, HC, CAP], BF16,
                            kind="ExternalInput") for b in range(NB)]
    xs_d = nc.dram_tensor("xs", [128, HC, SST], BF16, kind="ExternalInput")
    wr_d = nc.dram_tensor("wrT", [128, HC, E], F32, kind="ExternalInput")
    esel_d = nc.dram_tensor("esel", [128, E], F32, kind="ExternalInput")
    wg_d = nc.dram_tensor("wgB", [128, IC, HC, 128], BF16,
                          kind="ExternalInput")
    wu_d = nc.dram_tensor("wuB", [128, IC, HC, 128], BF16,
                          kind="ExternalInput")
    wgs_d = nc.dram_tensor("wgsB", [128, IC, HC, 128], BF16,
                           kind="ExternalInput")
    wus_d = nc.dram_tensor("wusB", [128, IC, HC, 128], BF16,
                           kind="ExternalInput")
    wd_d = nc.dram_tensor("wdB", [128, IC, H], BF16, kind="ExternalInput")
    wds_d = nc.dram_tensor("wdsB", [128, IC, H], BF16, kind="ExternalInput")
    sm_d = nc.dram_tensor("smB", [NB, 128, CB, 128], BF16,
                          kind="ExternalInput")
    y_d = nc.dram_tensor("y", [SST, H], F32, kind="ExternalOutput")

    with tile.TileContext(nc) as tc:
        with (
            tc.tile_pool(name="wres", bufs=1) as wres,
            tc.tile_pool(name="xgf", bufs=1) as xgfp,
            tc.tile_pool(name="wsh", bufs=8) as wshp,
            tc.tile_pool(name="act", bufs=1) as act,
            tc.tile_pool(name="small", bufs=2) as small,
            tc.tile_pool(name="sgp", bufs=2) as sgp,
            tc.tile_pool(name="htmp", bufs=3) as htmp,
            tc.tile_pool(name="osb", bufs=3) as osb,
            tc.tile_pool(name="fin", bufs=3) as fin,
            tc.tile_pool(name="yp", bufs=2) as ypool,
            tc.tile_pool(name="psA", bufs=1, space="PSUM") as psA,
            tc.tile_pool(name="psB", bufs=1, space="PSUM") as psB,
            tc.tile_pool(name="dram", bufs=1, space="DRAM") as dram,
        ):
            a2a_in = [dram.tile([CAP, H], BF16, tag=f"ai{b}", name=f"ai{b}")
                      for b in range(NB)]
            a2a_out = [dram.tile([CAP, H], BF16, tag=f"ao{b}", name=f"ao{b}")
                       for b in range(NB)]

            # ---- bulk unconditional loads (sync ring), in consumption order
            xgb = []
            t = act.tile([128, HC, CAP], BF16, tag="xgb0", name="xgb0")
            nc.sync.dma_start(t[:], xgb_d[0][:])
            xgb.append(t)
            wrT_sb = wres.tile([128, HC, E], F32, tag="wrT")
            nc.sync.dma_start(wrT_sb[:], wr_d[:])
            esel_sb = wres.tile([128, E], F32, tag="esel")
            nc.sync.dma_start(esel_sb[:], esel_d[:])
            ident8 = wres.tile([8, 8], F32, tag="ident8")
            make_identity(nc, ident8[:])

            wg_sb = wres.tile([128, IC, HC, 128], BF16, tag="wg")
            wu_sb = wres.tile([128, IC, HC, 128], BF16, tag="wu")
            wd_sb = wres.tile([128, IC, H], BF16, tag="wd")
            # small leading groups so sweep(0) can start early; all weights
            # precede the remaining activations so the sweep never starves
            groups = [(0, 1), (1, 3), (3, 6), (6, 9), (9, 11)]
            for i0, i1 in groups:
                for dst, src in ((wg_sb, wg_d), (wu_sb, wu_d), (wd_sb, wd_d)):
                    nc.sync.dma_start(dst[:, i0:i1], src[:, i0:i1])
            # f32 router activations (consumed by the router matmuls
            # interleaved into each sweep once the PE is warm)
            xgf = []
            for b in range(NB):
                t = xgfp.tile([128, HC, CAP], F32, tag=f"xgf{b}",
                              name=f"xgf{b}")
                nc.sync.dma_start(t[:], xg_d[b][:])
                xgf.append(t)
            t = act.tile([128, HC, CAP], BF16, tag="xgb1", name="xgb1")
            nc.sync.dma_start(t[:], xgb_d[1][:])
            xgb.append(t)
            xs_sb = act.tile([128, HC, SST], BF16, tag="xs")
            nc.sync.dma_start(xs_sb[:], xs_d[:])
            sm_sb = act.tile([128, NB, CB, 128], BF16, tag="sm")
            for b in range(NB):
                nc.sync.dma_start(sm_sb[:, b], sm_d[b])
            wds_sb = wres.tile([128, IC, H], BF16, tag="wds")
            for i0, i1 in ((0, 6), (6, 11)):
                nc.sync.dma_start(wds_sb[:, i0:i1], wds_d[:, i0:i1])
            # shared-expert g/u weights stream (pool-paced WAR waits are fine
            # at the tail of the sync ring)
            wsh = {}
            for it in range(IC):
                for nm, src in (("gs", wgs_d), ("us", wus_d)):
                    t = wshp.tile([128, HC, 128], BF16, tag="wsh",
                                  name=f"wsh_{nm}{it}")
                    nc.sync.dma_start(t[:], src[:, it])
                    wsh[(nm, it)] = t

            def router_cw(b):
                """Exact-f32 top-2 softmax combine weight for this core's
                expert, per gathered token slot.  Identical math to v4."""
                lgT_ps = psA.tile([8, CAP], F32, tag="g_ps", name=f"lgT{b}")
                for hc in range(HC):
                    nc.tensor.matmul(lgT_ps[:], wrT_sb[:, hc, :],
                                     xgf[b][:, hc, :],
                                     start=(hc == 0), stop=(hc == HC - 1))
                lgT_sb = small.tile([8, CAP], F32, tag="lgTs",
                                    name=f"lgTs{b}")
                nc.vector.tensor_copy(lgT_sb[:], lgT_ps[:])
                lg = small.tile([128, CB, E], F32, tag="lg", name=f"lg{b}")
                for m4 in range(CB):
                    ltr_ps = psA.tile([128, 8], F32, tag="u_ps",
                                      name=f"ltr{b}_{m4}")
                    nc.tensor.transpose(
                        ltr_ps[:], lgT_sb[:, m4 * 128:(m4 + 1) * 128],
                        ident8[:])
                    nc.vector.tensor_copy(lg[:, m4, :], ltr_ps[:])
                m1 = small.tile([128, CB, 1], F32, tag="m1", name=f"m1{b}")
                nc.vector.tensor_reduce(m1[:], lg[:], axis=mybir.AxisListType.X,
                                        op=OP.max)
                m1b = m1[:].to_broadcast([128, CB, E])
                is1 = small.tile([128, CB, E], F32, tag="is1", name=f"is1{b}")
                nc.vector.tensor_tensor(is1[:], lg[:], m1b, OP.is_ge)
                lgm = small.tile([128, CB, E], F32, tag="lgm", name=f"lgm{b}")
                nc.vector.scalar_tensor_tensor(
                    lgm[:], is1[:], -1e30, lg[:], op0=OP.mult, op1=OP.add)
                m2 = small.tile([128, CB, 1], F32, tag="m2", name=f"m2{b}")
                nc.vector.tensor_reduce(m2[:], lgm[:], axis=mybir.AxisListType.X,
                                        op=OP.max)
                dd = small.tile([128, CB, E], F32, tag="dd", name=f"dd{b}")
                nc.vector.tensor_tensor(dd[:], lg[:], m1b, OP.subtract)
                ee = small.tile([128, CB, E], F32, tag="ee", name=f"ee{b}")
                nc.scalar.activation(ee[:], dd[:], AF.Exp)
                d2 = small.tile([128, CB, 1], F32, tag="d2", name=f"d2{b}")
                nc.vector.tensor_tensor(d2[:], m2[:], m1[:], OP.subtract)
                e2 = small.tile([128, CB, 1], F32, tag="e2", name=f"e2{b}")
                nc.scalar.activation(e2[:], d2[:], AF.Exp)
                den = small.tile([128, CB, 1], F32, tag="den", name=f"den{b}")
                nc.vector.tensor_scalar_add(den[:], e2[:], 1.0)
                rden = small.tile([128, CB, 1], F32, tag="rden",
                                  name=f"rden{b}")
                nc.vector.reciprocal(rden[:], den[:])
                mask = small.tile([128, CB, E], F32, tag="mask",
                                  name=f"mask{b}")
                nc.vector.tensor_tensor(mask[:], lg[:],
                                        m2[:].to_broadcast([128, CB, E]),
                                        OP.is_ge)
                cwa = small.tile([128, CB, E], F32, tag="cwa", name=f"cwa{b}")
                nc.vector.tensor_tensor(cwa[:], ee[:], mask[:], OP.mult)
                nc.vector.tensor_tensor(cwa[:], cwa[:],
                                        rden[:].to_broadcast([128, CB, E]),
                                        OP.mult)
                esel_b = esel_sb[:].unsqueeze(1).to_broadcast([128, CB, E])
                nc.vector.tensor_tensor(cwa[:], cwa[:], esel_b, OP.mult)
                cwt = small.tile([128, CB, 1], F32, tag=f"cw{b}",
                                 name=f"cw{b}")
                nc.vector.tensor_reduce(cwt[:], cwa[:], axis=mybir.AxisListType.X,
                                        op=OP.add)
                return cwt

            cw_g = [None, None]

            def sweep(b):
                """g/u + down-proj (lagged one I-tile) for batch b.
                Router for batch b is interleaved after I-tile 0's g/u so
                the PE is already warm and nothing blocks sweep start."""
                ob = [psB.tile([128, 512], F32, tag=f"oA{j}", name=f"ob{b}_{j}")
                      for j in range(2 * CB)]
                h_prev = None

                def down(it, h0):
                    for m in range(CB):
                        for hn in range(HN):
                            nc.tensor.matmul(
                                ob[m * HN + hn][:],
                                h0[:, m * 128:(m + 1) * 128],
                                wd_sb[:, it, hn * 512:(hn + 1) * 512],
                                start=(it == 0), stop=(it == IC - 1))

                for it in range(IC):
                    g_ps = psA.tile([128, CAP], F32, tag="g_ps",
                                    name=f"g{b}_{it}")
                    u_ps = psA.tile([128, CAP], F32, tag="u_ps",
                                    name=f"u{b}_{it}")
                    # interleave g/u so consecutive matmuls hit different
                    # PSUM banks (same-bank accumulate chains serialize)
                    for hc in range(HC):
                        nc.tensor.matmul(g_ps[:], wg_sb[:, it, hc, :],
                                         xgb[b][:, hc, :],
                                         start=(hc == 0), stop=(hc == HC - 1))
                        nc.tensor.matmul(u_ps[:], wu_sb[:, it, hc, :],
                                         xgb[b][:, hc, :],
                                         start=(hc == 0), stop=(hc == HC - 1))
                    sg = sgp.tile([128, CAP], F32, tag="sg",
                                  name=f"sg{b}_{it}")
                    nc.scalar.activation(sg[:], g_ps[:], AF.Silu)
                    h0 = htmp.tile([128, CAP], BF16, tag="h0",
                                   name=f"h{b}_{it}")
                    nc.vector.tensor_tensor(h0[:], sg[:], u_ps[:], OP.mult)
                    if it == (7 if b == 0 else 2):
                        # after h0 so the psA WAR chain (silu/h0 reads) is
                        # already in the program when router reuses the tags;
                        # late enough that the f32 activations (queued behind
                        # all weights on the sync ring) have arrived
                        cw_g[b] = router_cw(b)
                    if h_prev is not None:
                        down(it - 1, h_prev)
                    h_prev = h0
                down(IC - 1, h_prev)
                # scale by combine weight, write compact bf16, exchange;
                # stores ride the scalar HWDGE ring (gpsimd SWDGE pays ~5us
                # of descriptor emission per 128-row store)
                for m in range(CB):
                    o_sb = osb.tile([128, H], BF16, tag="o_sb",
                                    name=f"osb{b}_{m}")
                    for hn in range(HN):
                        nc.vector.tensor_scalar_mul(
                            o_sb[:, hn * 512:(hn + 1) * 512],
                            ob[m * HN + hn][:], cw_g[b][:, m, :])
                    nc.scalar.dma_start(a2a_in[b][m * 128:(m + 1) * 128, :],
                                        o_sb[:])
                nc.gpsimd.collective_compute(
                    "AllToAll", OP.bypass,
                    replica_groups=[list(range(NCORES))],
                    ins=[a2a_in[b][:].opt()],
                    outs=[a2a_out[b][:].opt()],
                )

            sweep(0)
            sweep(1)

            # a2a output loads ride the tail of the sync ring: their waits on
            # collective completion cannot block any other engine's stream
            rc = {}
            for b in range(NB):
                for rk in range(CB):
                    t = fin.tile([128, H], BF16, tag="rc", name=f"rc{b}_{rk}")
                    nc.sync.dma_start(
                        t[:], a2a_out[b][rk * 128:(rk + 1) * 128, :])
                    rc[(b, rk)] = t

            # ---- merge batch 0 right after sweep(1): keeps the PE warm and
            # runs long before anything needs it ----
            y_ps = {}

            def merge(b):
                for hn in range(HN):
                    hsl = slice(hn * 512, (hn + 1) * 512)
                    yp = psB.tile([128, 512], F32, tag=f"oA{2 * b + hn}",
                                  name=f"y_ps{b}_{hn}")
                    for rk in range(CB):
                        nc.tensor.matmul(yp[:], sm_sb[:, b, rk, :],
                                         rc[(b, rk)][:, hsl],
                                         start=(rk == 0), stop=(rk == CB - 1))
                    y_ps[(b, hn)] = yp

            merge(0)

            # ---- shared expert g/u (covers a2a latency) ----
            hs_sb = act.tile([128, IC, SST], BF16, tag="hs")
            for it in range(IC):
                gs_ps = psA.tile([128, SST], F32, tag="g_ps",
                                 name=f"gs_{it}")
                for hc in range(HC):
                    nc.tensor.matmul(gs_ps[:], wsh[("gs", it)][:, hc, :],
                                     xs_sb[:, hc, :],
                                     start=(hc == 0), stop=(hc == HC - 1))
                sgs = sgp.tile([128, SST], F32, tag="sgs", name=f"sgs_{it}")
                nc.scalar.activation(sgs[:], gs_ps[:], AF.Silu)
                us_ps = psA.tile([128, SST], F32, tag="u_ps",
                                 name=f"us_{it}")
                for hc in range(HC):
                    nc.tensor.matmul(us_ps[:], wsh[("us", it)][:, hc, :],
                                     xs_sb[:, hc, :],
                                     start=(hc == 0), stop=(hc == HC - 1))
                nc.vector.tensor_tensor(hs_sb[:, it, :], sgs[:], us_ps[:],
                                        OP.mult)

            # ---- shared down-proj ----
            s_out = act.tile([128, NB, H], F32, tag="s_out")
            for hn in range(HN):
                hsl = slice(hn * 512, (hn + 1) * 512)
                s_ps = [psA.tile([128, 512], F32, tag=("g_ps", "u_ps")[m],
                                 name=f"s_ps{m}_{hn}") for m in range(NB)]
                for it in range(IC):
                    for m in range(NB):
                        nc.tensor.matmul(s_ps[m][:],
                                         hs_sb[:, it, m * 128:(m + 1) * 128],
                                         wds_sb[:, it, hsl],
                                         start=(it == 0), stop=(it == IC - 1))
                for m in range(NB):
                    nc.scalar.copy(s_out[:, m, hsl], s_ps[m][:])

            # ---- finalize batch 0, then batch 1 ----
            def finalize(b):
                y_sb = ypool.tile([128, H], F32, tag="y_sb", name=f"ysb{b}")
                for hn in range(HN):
                    hsl = slice(hn * 512, (hn + 1) * 512)
                    nc.vector.tensor_tensor(y_sb[:, hsl], y_ps[(b, hn)][:],
                                            s_out[:, b, hsl], OP.add)
                nc.scalar.dma_start(y_d[b * 128:(b + 1) * 128, :], y_sb[:])

            merge(1)
            finalize(0)
            finalize(1)

    nc.compile()
    return nc


def _get_nc():
    if "nc" not in _CACHE:
        _CACHE["nc"] = _build()
    return _CACHE["nc"]


def _reblock_gu(w):
    # [H, I] -> [128, IC, HC, 128] bf16: [q, it, hc, p] = w[hc*128+q, it*128+p]
    return np.ascontiguousarray(
        w.reshape(HC, 128, IC, 128).transpose(1, 2, 0, 3)).astype(BF16NP)


def _reblock_d(w):
    # [I, H] -> [128, IC, H] bf16: [k, it, h] = w[it*128+k, h]
    return np.ascontiguousarray(
        w.reshape(IC, 128, H).transpose(1, 0, 2)).astype(BF16NP)


def _pack_pm(a):
    # [H, N] -> [128, HC, N]: [p, hc, n] = a[hc*128+p, n]
    return np.ascontiguousarray(a.reshape(HC, 128, -1).transpose(1, 0, 2))


def make_in_maps(x, w_router, wg, wu, wd, wg_s, wu_s, wd_s):
    xf = x.reshape(T, H)
    xT = np.ascontiguousarray(xf.T)

    # host-side dispatch plan: top-2 selection per token
    logits = xf @ w_router.T                      # [T, E]
    part = np.argpartition(-logits, 2, axis=1)[:, :2]   # top-2 expert ids

    wrT = _pack_pm(np.ascontiguousarray(w_router.T))    # [128, HC, E] f32
    wgsB = _reblock_gu(wg_s)
    wusB = _reblock_gu(wu_s)
    wdsB = _reblock_d(wd_s)

    # dispatch tables: for (batch, expert) owner-sorted slot assignment
    gsel = np.zeros((NB, NCORES, CAP), np.int64)      # gathered token ids
    smT = np.zeros((NB, NCORES, CAP, 128), np.float32)  # receiver merge mats
    for b in range(NB):
        sel_b = part[b * TB:(b + 1) * TB]
        for e in range(NCORES):
            sel = np.where((sel_b == e).any(axis=1))[0]   # tokens picking e
            gsel[b, e, :] = b * TB                        # pad default
            for o in range(NCORES):
                grp = sel[(sel // 128) == o]
                n = len(grp)
                assert n <= SLOT, f"slot overflow: {n} > {SLOT}"
                gsel[b, e, o * SLOT:o * SLOT + n] = b * TB + grp
                # receiver o's merge matrix: recv row e*SLOT+k -> local row
                smT[b, o, e * SLOT + np.arange(n), grp - o * 128] = 1.0
    in_maps = []
    for c in range(NCORES):
        xsT = np.concatenate([xT[:, c * 128:(c + 1) * 128],
                              xT[:, TB + c * 128:TB + (c + 1) * 128]], axis=1)
        m = {
            "xs": _pack_pm(xsT).astype(BF16NP),
            "wrT": wrT,
            "wgB": _reblock_gu(wg[c]),
            "wuB": _reblock_gu(wu[c]),
            "wdB": _reblock_d(wd[c]),
            "wgsB": wgsB,
            "wusB": wusB,
            "wdsB": wdsB,
        }
        esel = np.zeros((128, E), np.float32)
        esel[:, c] = 1.0
        m["esel"] = esel
        for b in range(NB):
            xgc = _pack_pm(np.ascontiguousarray(xT[:, gsel[b, c]]))
            m[f"xg{b}"] = xgc
            m[f"xgb{b}"] = xgc.astype(BF16NP)
        m["smB"] = np.ascontiguousarray(
            smT[:, c].reshape(NB, CB, 128, 128).transpose(0, 2, 1, 3)
        ).astype(BF16NP)
        in_maps.append(m)
    return in_maps


def kernel(x, w_router, wg, wu, wd, wg_s, wu_s, wd_s):
    x = np.asarray(x, dtype=np.float32)
    w_router = np.asarray(w_router, dtype=np.float32)
    wg = np.asarray(wg, dtype=np.float32)
    wu = np.asarray(wu, dtype=np.float32)
    wd = np.asarray(wd, dtype=np.float32)
    wg_s = np.asarray(wg_s, dtype=np.float32)
    wu_s = np.asarray(wu_s, dtype=np.float32)
    wd_s = np.asarray(wd_s, dtype=np.float32)

    nc = _get_nc()
    in_maps = make_in_maps(x, w_router, wg, wu, wd, wg_s, wu_s, wd_s)
    res = run_bass_kernel_spmd(nc, in_maps, list(range(NCORES)))

    y = np.zeros((T, H), np.float32)
    for c in range(NCORES):
        yc = res.results[c]["y"]
        for b in range(NB):
            y[b * TB + c * 128: b * TB + (c + 1) * 128] = \
                yc[b * 128:(b + 1) * 128]
    return y.reshape(B, S, H)
